# revision 21
# baseline (speedup 1.0000x reference)
"""Trainium2 Bass kernel for nn_DepthwiseTemporalConv.

Reference semantics (derived, validated exactly vs the oracle):
  x: (4, 256, 64, 32, 32) f32, weight: (256, 1, 64) f32
  x_raw = x.view(4096, 256, 64)                       # raw row-major reinterpretation
  y_raw[n, c, t'] = sum_{t>=t'} w[c, t-t'] * x_raw[n, c, t]
                  = (x_raw[n, c, :] @ U_c)[t'], U_c[t, t'] = w[c, t-t'] (lower-tri Toeplitz)
  out.view(4, 256, 64, 1024)[b, c, t', m] = y_raw[b*1024 + m, c, t']

Strategy: shard n = b*1024 + m over 8 cores (512 blocks each, contiguous 32 MiB
slices of x). Per core: DMA natural [n, (c,t)] tiles -> PE transpose (fp32 via
identity, exact) to [(c-pair, t), n] -> per-channel Toeplitz matmuls
(tile_position quadrant packing, both channels of a pair concurrent) -> PSUM ->
SBUF staging -> contiguous DMA out in [(c, t'), n] layout.

Matmul modes:
  bf16s (default): hi/lo bf16 split, 3 accumulating matmuls per channel
        (Wh.xh + Wh.xl + Wl.xh), products exact into fp32 PSUM; ~5e-6 rel err
        at full PE rate (1 cyc/row).
  f32:  exact fp32 matmuls (4 cyc/row), ~2e-7 rel err.
  f32r: TF32-like fast mode (f32r end-to-end path), ~1.4e-4 rel err.
"""
import numpy as np

B, C, T, H, W = 4, 256, 64, 32, 32
K = 64
NCORES = 8
NB = B * H * W          # 4096 raw blocks
NPC = NB // NCORES      # 512 blocks per core
CT = C * T              # 16384

N_TILES = NPC // 128    # 4 n-tiles of 128 per core
C_CHUNK = 32            # channels per chunk (16 pairs)
N_CHUNKS = C // C_CHUNK  # 8 chunks
PAIRS_PER_CHUNK = C_CHUNK // 2  # 16

_cache = {}
COMBINED_IN = True

# test-harness knobs (the grading harness just calls kernel(**inputs))
MODE = "bf16sbd"        # "bf16sbd" | "bf16s" | "f32" | "f32r"
TRACE = False
LAST_RESULT = None


def _build_nc(mode: str = "bf16s", loops: int = 1, *, xin_bufs=3, xt_bufs=6, stage_bufs=2, pst_bufs=4, psy_bufs=4, c_chunk=C_CHUNK):
    import concourse.bass as bass
    import concourse.bacc as bacc
    import concourse.tile as tile
    from concourse import mybir
    from concourse.masks import make_identity

    f32 = mybir.dt.float32
    f32r = mybir.dt.float32r
    bf16 = mybir.dt.bfloat16
    # f32r mode declares the whole x/w path float32r (identical 32-bit layout;
    # numpy side stays float32) so the BIR verifier sees consistent dtypes.
    xdt = f32r if mode == "f32r" else f32
    wdt = bf16 if mode == "bf16s" else xdt

    nc = bacc.Bacc("TRN2", target_bir_lowering=False, debug=False)

    # Per-core shard of x, viewed as [512 n-blocks, 16384 (c,t)]
    x_d = nc.dram_tensor("x", [NPC, CT], xdt, kind="ExternalInput")
    # Toeplitz weights, channel pairs stacked: rows 0:64 = even channel U
    # (t rows), rows 64:128 = odd channel U; pair j at cols [64j, 64j+64)
    if mode == "bf16sbd":
        # block-diag weights: per pair [128, 128] = diag(U_even, U_odd)
        w_d = nc.dram_tensor("w", [128, (C // 2) * 128], bf16,
                             kind="ExternalInput")
        wl_d = nc.dram_tensor("wl", [128, (C // 2) * 128], bf16,
                              kind="ExternalInput")
    else:
        w_d = nc.dram_tensor("w", [128, (C // 2) * K], wdt, kind="ExternalInput")
    if mode in ("bf16s", "bf16sbdc"):
        wl_d = nc.dram_tensor("wl", [128, (C // 2) * K], bf16,
                              kind="ExternalInput")
    # Per-core output: [(c, t'), n] = [256*64, 512]
    y_d = nc.dram_tensor("y", [C * T, NPC], f32, kind="ExternalOutput")

    n_chunks = C // c_chunk
    pairs_per_chunk = c_chunk // 2
    with tile.TileContext(nc) as tc:
        with (
            tc.tile_pool(name="const", bufs=1) as const_pool,
            tc.tile_pool(name="xin", bufs=xin_bufs) as x_pool,
            tc.tile_pool(name="xt", bufs=xt_bufs) as xt_pool,
            tc.tile_pool(name="stage", bufs=stage_bufs) as stage_pool,
            tc.tile_pool(name="wbd", bufs=3) as wbd_pool,
            tc.tile_pool(name="pst", bufs=pst_bufs, space="PSUM") as pst_pool,
            tc.tile_pool(name="psy", bufs=psy_bufs, space="PSUM") as psy_pool,
        ):
            ident = const_pool.tile([128, 128], xdt)
            make_identity(nc, ident)
            if mode != "bf16sbd":
                w_sb = const_pool.tile([128, (C // 2) * K], wdt)
                nc.sync.dma_start(out=w_sb, in_=w_d.ap())
            if mode in ("bf16s", "bf16sbdc"):
                wl_sb = const_pool.tile([128, (C // 2) * K], bf16)
                nc.sync.dma_start(out=wl_sb, in_=wl_d.ap())

            for cc in range(n_chunks * loops):
                cc = cc % n_chunks
                # Load the 4 n-tiles for this channel chunk:
                # [128 n, 32 ch * 64 t] each; contiguous 8 KiB per partition.
                ck = c_chunk * K
                if COMBINED_IN:
                    # one DMA for all 4 n-tiles: dst [p, (k, c)], src
                    # x[(k*128+p), ccols] via 3D AP
                    xin_big = x_pool.tile([128, N_TILES * ck], xdt, tag="xinb")
                    src = bass.AP(
                        tensor=x_d,
                        offset=cc * ck,
                        ap=[[CT, 128], [128 * CT, N_TILES], [1, ck]],
                    )
                    nc.sync.dma_start(out=xin_big, in_=src)
                    xk = [xin_big[:, k * ck:(k + 1) * ck]
                          for k in range(N_TILES)]
                else:
                    xk = []
                    for k in range(N_TILES):
                        xt_in = x_pool.tile([128, ck], xdt, tag="xin")
                        nc.sync.dma_start(
                            out=xt_in,
                            in_=x_d.ap()[k * 128:(k + 1) * 128,
                                         cc * ck:(cc + 1) * ck],
                        )
                        xk.append(xt_in)

                stage = stage_pool.tile([128, pairs_per_chunk * NPC], f32)
                if mode == "bf16sbd":
                    wcs = pairs_per_chunk * 128
                    wbd_h = wbd_pool.tile([128, wcs], bf16, tag="wbdh")
                    wbd_l = wbd_pool.tile([128, wcs], bf16, tag="wbdl")
                    nc.sync.dma_start(out=wbd_h,
                                      in_=w_d.ap()[:, cc * wcs:(cc + 1) * wcs])
                    nc.sync.dma_start(out=wbd_l,
                                      in_=wl_d.ap()[:, cc * wcs:(cc + 1) * wcs])
                elif mode == "bf16sbdc":
                    # construct block-diag on-chip from compact weights:
                    # memset zeros (Pool), then partition-aligned strided
                    # copies of the diagonal quadrants
                    wcs = pairs_per_chunk * 128
                    wbd_h = wbd_pool.tile([128, wcs], bf16, tag="wbdh")
                    wbd_l = wbd_pool.tile([128, wcs], bf16, tag="wbdl")
                    ccols = slice(cc * pairs_per_chunk * K,
                                  (cc + 1) * pairs_per_chunk * K)
                    for wbd, wsrc in ((wbd_h, w_sb), (wbd_l, wl_sb)):
                        nc.gpsimd.memset(wbd[:], 0.0)
                        dst = wbd[:].rearrange("p (j c) -> p j c", c=128)
                        srcv = wsrc[:, ccols].rearrange("p (j c) -> p j c", c=K)
                        nc.vector.tensor_copy(dst[0:64, :, 0:64], srcv[0:64])
                        nc.scalar.copy(dst[64:128, :, 64:128], srcv[64:128])

                for j in range(pairs_per_chunk):
                    pair = cc * pairs_per_chunk + j  # global pair index
                    # Transpose the pair's [128 n, 128 (c0,c1 t)] slices of
                    # the 4 n-tiles into one [(c0 t | c1 t), 512 n] tile.
                    xt_ps = pst_pool.tile([128, NPC], xdt)
                    for k in range(N_TILES):
                        nc.tensor.transpose(
                            xt_ps[:, k * 128:(k + 1) * 128],
                            xk[k][:, j * 128:(j + 1) * 128],
                            ident[:],
                        )

                    y_ps = psy_pool.tile([128, NPC], f32)
                    wcols = slice(pair * K, (pair + 1) * K)
                    if mode in ("bf16sbd", "bf16sbdc"):
                        xh_sb = xt_pool.tile([128, NPC], bf16, tag="xh")
                        xl_sb = xt_pool.tile([128, NPC], bf16, tag="xl")
                        nc.scalar.copy(xh_sb[:], xt_ps[:])
                        nc.vector.tensor_sub(xl_sb[:], xt_ps[:], xh_sb[:])
                        jc = slice(j * 128, (j + 1) * 128)
                        for i, (wsrc, xsrc) in enumerate(
                                ((wbd_h, xh_sb), (wbd_h, xl_sb), (wbd_l, xh_sb))):
                            nc.tensor.matmul(
                                y_ps[:], wsrc[:, jc], xsrc[:],
                                start=(i == 0), stop=(i == 2),
                            )
                    elif mode == "bf16s":
                        # split during PSUM evacuation: xh = bf16(xt) on ACT,
                        # xl = bf16(xt - xh) on DVE
                        xh_sb = xt_pool.tile([128, NPC], bf16, tag="xh")
                        xl_sb = xt_pool.tile([128, NPC], bf16, tag="xl")
                        nc.scalar.copy(xh_sb[:], xt_ps[:])
                        nc.vector.tensor_sub(xl_sb[:], xt_ps[:], xh_sb[:])
                        # per channel quadrant: Wh.xh + Wh.xl + Wl.xh,
                        # interleaved so the two quadrants overlap on the PE
                        passes = ((w_sb, xh_sb), (w_sb, xl_sb), (wl_sb, xh_sb))
                        for i, (wsrc, xsrc) in enumerate(passes):
                            for lo, hi in ((0, 64), (64, 128)):
                                nc.tensor.matmul(
                                    y_ps[lo:hi, :], wsrc[lo:hi, wcols],
                                    xsrc[lo:hi, :],
                                    start=(i == 0), stop=(i == 2),
                                    tile_position=(lo, lo),
                                )
                    else:
                        xt_sb = xt_pool.tile([128, NPC], xdt, tag="xt")
                        nc.scalar.copy(xt_sb[:], xt_ps[:])
                        for lo, hi in ((0, 64), (64, 128)):
                            nc.tensor.matmul(
                                y_ps[lo:hi, :], w_sb[lo:hi, wcols],
                                xt_sb[lo:hi, :],
                                start=True, stop=True, tile_position=(lo, lo),
                            )
                    # y evacuation, alternating engines to balance load
                    dst = stage[:, j * NPC:(j + 1) * NPC]
                    if mode.startswith("bf16s") and j % 2 == 0:
                        nc.scalar.copy(dst, y_ps[:])
                    else:
                        nc.vector.tensor_copy(dst, y_ps[:])

                # stage[p, j*512+m] maps to DRAM element
                # cc*2048*512 + j*(128*512) + p*512 + m  (channel-pair rows
                # are contiguous since (2j)*64+p covers p in [0,128)).
                out_ap = bass.AP(
                    tensor=y_d,
                    offset=cc * c_chunk * K * NPC,
                    ap=[[NPC, 128], [128 * NPC, pairs_per_chunk], [1, NPC]],
                )
                nc.sync.dma_start(out=out_ap, in_=stage[:])
    nc.finalize()
    return nc


def _toeplitz_weights(weight: np.ndarray) -> np.ndarray:
    """Build [128, (C//2)*K] paired lower-triangular Toeplitz weight matrix."""
    w = weight.reshape(C, K).astype(np.float32)
    t = np.arange(K)
    idx = t[:, None] - t[None, :]            # [t, t'] = t - t'
    mask = idx >= 0
    U = w[:, np.clip(idx, 0, K - 1)] * mask  # (C, K, K): U[c, t, t'] = w[c, t-t']
    Wp = np.empty((128, (C // 2) * K), dtype=np.float32)
    # pair j: even channel 2j -> rows 0:64, odd channel 2j+1 -> rows 64:128
    Wp[0:64] = U[0::2].transpose(1, 0, 2).reshape(K, -1)
    Wp[64:128] = U[1::2].transpose(1, 0, 2).reshape(K, -1)
    return Wp


def kernel(x: np.ndarray, weight: np.ndarray) -> np.ndarray:
    import ml_dtypes
    from concourse.bass_utils import run_bass_kernel_spmd

    if MODE not in _cache:
        _cache[MODE] = _build_nc(mode=MODE)
    nc = _cache[MODE]

    x = np.ascontiguousarray(x, dtype=np.float32)
    Wp = _toeplitz_weights(np.asarray(weight))
    if MODE == "bf16sbd":
        Wbd = np.zeros((128, (C // 2) * 128), np.float32)
        for j in range(C // 2):
            Wbd[0:64, j * 128:j * 128 + 64] = Wp[0:64, j * K:(j + 1) * K]
            Wbd[64:128, j * 128 + 64:(j + 1) * 128] = Wp[64:128, j * K:(j + 1) * K]
        Wbdh = Wbd.astype(ml_dtypes.bfloat16)
        Wbdl = (Wbd - Wbdh.astype(np.float32)).astype(ml_dtypes.bfloat16)

    x_raw = x.reshape(NB, CT)
    in_maps = []
    for k in range(NCORES):
        m = {"x": x_raw[k * NPC:(k + 1) * NPC]}
        if MODE == "bf16sbd":
            m["w"] = Wbdh
            m["wl"] = Wbdl
        elif MODE in ("bf16s", "bf16sbdc"):
            Wh = Wp.astype(ml_dtypes.bfloat16)
            m["w"] = Wh
            m["wl"] = (Wp - Wh.astype(np.float32)).astype(ml_dtypes.bfloat16)
        else:
            m["w"] = Wp
        in_maps.append(m)
    res = run_bass_kernel_spmd(nc, in_maps, core_ids=list(range(NCORES)),
                               trace=TRACE)
    global LAST_RESULT
    LAST_RESULT = res

    # Assemble: core k holds out_v[b = k//2, :, :, m-half]
    out_v = np.empty((B, C, T, H * W), dtype=np.float32)
    for k in range(NCORES):
        yk = res.results[k]["y"].reshape(C, T, NPC)
        b, half = divmod(k, 2)
        out_v[b, :, :, half * NPC:(half + 1) * NPC] = yk
    return out_v.reshape(B, C, T, H, W)


if __name__ == "__main__":
    x = np.load("/tmp/x.npy")
    w = np.load("/tmp/w.npy")
    out = kernel(x, w)
    exp = np.load("/tmp/expected.npy")
    denom = np.abs(exp).max()
    print("max abs err:", np.abs(out - exp).max(), "absmax:", denom)
    print("rel:", np.abs(out - exp).max() / denom)


# revision 23
# speedup vs baseline: 1.0311x; 1.0311x over previous
"""Trainium2 Bass kernel for nn_DepthwiseTemporalConv.

Reference semantics (derived, validated exactly vs the oracle):
  x: (4, 256, 64, 32, 32) f32, weight: (256, 1, 64) f32
  x_raw = x.view(4096, 256, 64)                       # raw row-major reinterpretation
  y_raw[n, c, t'] = sum_{t>=t'} w[c, t-t'] * x_raw[n, c, t]
                  = (x_raw[n, c, :] @ U_c)[t'], U_c[t, t'] = w[c, t-t'] (lower-tri Toeplitz)
  out.view(4, 256, 64, 1024)[b, c, t', m] = y_raw[b*1024 + m, c, t']

Strategy: shard n = b*1024 + m over 8 cores (512 blocks each, contiguous 32 MiB
slices of x). Per core: DMA natural [n, (c,t)] tiles -> PE transpose (fp32 via
identity, exact) to [(c-pair, t), n] -> per-channel Toeplitz matmuls
(tile_position quadrant packing, both channels of a pair concurrent) -> PSUM ->
SBUF staging -> contiguous DMA out in [(c, t'), n] layout.

Matmul modes:
  bf16s (default): hi/lo bf16 split, 3 accumulating matmuls per channel
        (Wh.xh + Wh.xl + Wl.xh), products exact into fp32 PSUM; ~5e-6 rel err
        at full PE rate (1 cyc/row).
  f32:  exact fp32 matmuls (4 cyc/row), ~2e-7 rel err.
  f32r: TF32-like fast mode (f32r end-to-end path), ~1.4e-4 rel err.
"""
import numpy as np

B, C, T, H, W = 4, 256, 64, 32, 32
K = 64
NCORES = 8
NB = B * H * W          # 4096 raw blocks
NPC = NB // NCORES      # 512 blocks per core
CT = C * T              # 16384

N_TILES = NPC // 128    # 4 n-tiles of 128 per core
C_CHUNK = 32            # channels per chunk (16 pairs)
N_CHUNKS = C // C_CHUNK  # 8 chunks
PAIRS_PER_CHUNK = C_CHUNK // 2  # 16

_cache = {}
COMBINED_IN = True
OUT_SPLIT = 16

# test-harness knobs (the grading harness just calls kernel(**inputs))
MODE = "bf16sbd"        # "bf16sbd" | "bf16s" | "f32" | "f32r"
TRACE = False
LAST_RESULT = None


def _build_nc(mode: str = "bf16s", loops: int = 1, *, xin_bufs=3, xt_bufs=6, stage_bufs=2, pst_bufs=4, psy_bufs=4, c_chunk=C_CHUNK):
    import concourse.bass as bass
    import concourse.bacc as bacc
    import concourse.tile as tile
    from concourse import mybir
    from concourse.masks import make_identity

    f32 = mybir.dt.float32
    f32r = mybir.dt.float32r
    bf16 = mybir.dt.bfloat16
    # f32r mode declares the whole x/w path float32r (identical 32-bit layout;
    # numpy side stays float32) so the BIR verifier sees consistent dtypes.
    xdt = f32r if mode == "f32r" else f32
    wdt = bf16 if mode == "bf16s" else xdt

    nc = bacc.Bacc("TRN2", target_bir_lowering=False, debug=False)

    # Per-core shard of x, viewed as [512 n-blocks, 16384 (c,t)]
    x_d = nc.dram_tensor("x", [NPC, CT], xdt, kind="ExternalInput")
    # Toeplitz weights, channel pairs stacked: rows 0:64 = even channel U
    # (t rows), rows 64:128 = odd channel U; pair j at cols [64j, 64j+64)
    if mode == "bf16sbd":
        # block-diag weights: per pair [128, 128] = diag(U_even, U_odd)
        w_d = nc.dram_tensor("w", [128, (C // 2) * 128], bf16,
                             kind="ExternalInput")
        wl_d = nc.dram_tensor("wl", [128, (C // 2) * 128], bf16,
                              kind="ExternalInput")
    else:
        w_d = nc.dram_tensor("w", [128, (C // 2) * K], wdt, kind="ExternalInput")
    if mode in ("bf16s", "bf16sbdc"):
        wl_d = nc.dram_tensor("wl", [128, (C // 2) * K], bf16,
                              kind="ExternalInput")
    # Per-core output: [(c, t'), n] = [256*64, 512]
    y_d = nc.dram_tensor("y", [C * T, NPC], f32, kind="ExternalOutput")

    n_chunks = C // c_chunk
    pairs_per_chunk = c_chunk // 2
    with tile.TileContext(nc) as tc:
        with (
            tc.tile_pool(name="const", bufs=1) as const_pool,
            tc.tile_pool(name="xin", bufs=xin_bufs) as x_pool,
            tc.tile_pool(name="xt", bufs=xt_bufs) as xt_pool,
            tc.tile_pool(name="stage", bufs=stage_bufs) as stage_pool,
            tc.tile_pool(name="wbd", bufs=3) as wbd_pool,
            tc.tile_pool(name="pst", bufs=pst_bufs, space="PSUM") as pst_pool,
            tc.tile_pool(name="psy", bufs=psy_bufs, space="PSUM") as psy_pool,
        ):
            ident = const_pool.tile([128, 128], xdt)
            make_identity(nc, ident)
            if mode != "bf16sbd":
                w_sb = const_pool.tile([128, (C // 2) * K], wdt)
                nc.sync.dma_start(out=w_sb, in_=w_d.ap())
            if mode in ("bf16s", "bf16sbdc"):
                wl_sb = const_pool.tile([128, (C // 2) * K], bf16)
                nc.sync.dma_start(out=wl_sb, in_=wl_d.ap())

            for cc in range(n_chunks * loops):
                cc = cc % n_chunks
                # Load the 4 n-tiles for this channel chunk:
                # [128 n, 32 ch * 64 t] each; contiguous 8 KiB per partition.
                ck = c_chunk * K
                if COMBINED_IN:
                    # one DMA for all 4 n-tiles: dst [p, (k, c)], src
                    # x[(k*128+p), ccols] via 3D AP
                    xin_big = x_pool.tile([128, N_TILES * ck], xdt, tag="xinb")
                    src = bass.AP(
                        tensor=x_d,
                        offset=cc * ck,
                        ap=[[CT, 128], [128 * CT, N_TILES], [1, ck]],
                    )
                    nc.sync.dma_start(out=xin_big, in_=src)
                    xk = [xin_big[:, k * ck:(k + 1) * ck]
                          for k in range(N_TILES)]
                else:
                    xk = []
                    for k in range(N_TILES):
                        xt_in = x_pool.tile([128, ck], xdt, tag="xin")
                        nc.sync.dma_start(
                            out=xt_in,
                            in_=x_d.ap()[k * 128:(k + 1) * 128,
                                         cc * ck:(cc + 1) * ck],
                        )
                        xk.append(xt_in)

                stage = stage_pool.tile([128, pairs_per_chunk * NPC], f32)
                if mode == "bf16sbd":
                    wcs = pairs_per_chunk * 128
                    wbd_h = wbd_pool.tile([128, wcs], bf16, tag="wbdh")
                    wbd_l = wbd_pool.tile([128, wcs], bf16, tag="wbdl")
                    nc.sync.dma_start(out=wbd_h,
                                      in_=w_d.ap()[:, cc * wcs:(cc + 1) * wcs])
                    nc.sync.dma_start(out=wbd_l,
                                      in_=wl_d.ap()[:, cc * wcs:(cc + 1) * wcs])
                elif mode == "bf16sbdc":
                    # construct block-diag on-chip from compact weights:
                    # memset zeros (Pool), then partition-aligned strided
                    # copies of the diagonal quadrants
                    wcs = pairs_per_chunk * 128
                    wbd_h = wbd_pool.tile([128, wcs], bf16, tag="wbdh")
                    wbd_l = wbd_pool.tile([128, wcs], bf16, tag="wbdl")
                    ccols = slice(cc * pairs_per_chunk * K,
                                  (cc + 1) * pairs_per_chunk * K)
                    for wbd, wsrc in ((wbd_h, w_sb), (wbd_l, wl_sb)):
                        nc.gpsimd.memset(wbd[:], 0.0)
                        dst = wbd[:].rearrange("p (j c) -> p j c", c=128)
                        srcv = wsrc[:, ccols].rearrange("p (j c) -> p j c", c=K)
                        nc.vector.tensor_copy(dst[0:64, :, 0:64], srcv[0:64])
                        nc.scalar.copy(dst[64:128, :, 64:128], srcv[64:128])

                for j in range(pairs_per_chunk):
                    pair = cc * pairs_per_chunk + j  # global pair index
                    # Transpose the pair's [128 n, 128 (c0,c1 t)] slices of
                    # the 4 n-tiles into one [(c0 t | c1 t), 512 n] tile.
                    xt_ps = pst_pool.tile([128, NPC], xdt)
                    for k in range(N_TILES):
                        nc.tensor.transpose(
                            xt_ps[:, k * 128:(k + 1) * 128],
                            xk[k][:, j * 128:(j + 1) * 128],
                            ident[:],
                        )

                    y_ps = psy_pool.tile([128, NPC], f32)
                    wcols = slice(pair * K, (pair + 1) * K)
                    if mode in ("bf16sbd", "bf16sbdc"):
                        xh_sb = xt_pool.tile([128, NPC], bf16, tag="xh")
                        xl_sb = xt_pool.tile([128, NPC], bf16, tag="xl")
                        nc.scalar.copy(xh_sb[:], xt_ps[:])
                        nc.vector.tensor_sub(xl_sb[:], xt_ps[:], xh_sb[:])
                        jc = slice(j * 128, (j + 1) * 128)
                        for i, (wsrc, xsrc) in enumerate(
                                ((wbd_h, xh_sb), (wbd_h, xl_sb), (wbd_l, xh_sb))):
                            nc.tensor.matmul(
                                y_ps[:], wsrc[:, jc], xsrc[:],
                                start=(i == 0), stop=(i == 2),
                            )
                    elif mode == "bf16s":
                        # split during PSUM evacuation: xh = bf16(xt) on ACT,
                        # xl = bf16(xt - xh) on DVE
                        xh_sb = xt_pool.tile([128, NPC], bf16, tag="xh")
                        xl_sb = xt_pool.tile([128, NPC], bf16, tag="xl")
                        nc.scalar.copy(xh_sb[:], xt_ps[:])
                        nc.vector.tensor_sub(xl_sb[:], xt_ps[:], xh_sb[:])
                        # per channel quadrant: Wh.xh + Wh.xl + Wl.xh,
                        # interleaved so the two quadrants overlap on the PE
                        passes = ((w_sb, xh_sb), (w_sb, xl_sb), (wl_sb, xh_sb))
                        for i, (wsrc, xsrc) in enumerate(passes):
                            for lo, hi in ((0, 64), (64, 128)):
                                nc.tensor.matmul(
                                    y_ps[lo:hi, :], wsrc[lo:hi, wcols],
                                    xsrc[lo:hi, :],
                                    start=(i == 0), stop=(i == 2),
                                    tile_position=(lo, lo),
                                )
                    else:
                        xt_sb = xt_pool.tile([128, NPC], xdt, tag="xt")
                        nc.scalar.copy(xt_sb[:], xt_ps[:])
                        for lo, hi in ((0, 64), (64, 128)):
                            nc.tensor.matmul(
                                y_ps[lo:hi, :], w_sb[lo:hi, wcols],
                                xt_sb[lo:hi, :],
                                start=True, stop=True, tile_position=(lo, lo),
                            )
                    # y evacuation, alternating engines to balance load
                    dst = stage[:, j * NPC:(j + 1) * NPC]
                    if mode.startswith("bf16s") and j % 2 == 0:
                        nc.scalar.copy(dst, y_ps[:])
                    else:
                        nc.vector.tensor_copy(dst, y_ps[:])

                # stage[p, j*512+m] maps to DRAM element
                # cc*2048*512 + j*(128*512) + p*512 + m  (channel-pair rows
                # are contiguous since (2j)*64+p covers p in [0,128)).
                hp = pairs_per_chunk // 2
                for h in range(OUT_SPLIT):
                    ph = pairs_per_chunk // OUT_SPLIT
                    out_ap = bass.AP(
                        tensor=y_d,
                        offset=(cc * c_chunk * K + h * ph * 128) * NPC,
                        ap=[[NPC, 128], [128 * NPC, ph], [1, NPC]],
                    )
                    nc.sync.dma_start(
                        out=out_ap,
                        in_=stage[:, h * ph * NPC:(h + 1) * ph * NPC])
    nc.finalize()
    return nc


def _toeplitz_weights(weight: np.ndarray) -> np.ndarray:
    """Build [128, (C//2)*K] paired lower-triangular Toeplitz weight matrix."""
    w = weight.reshape(C, K).astype(np.float32)
    t = np.arange(K)
    idx = t[:, None] - t[None, :]            # [t, t'] = t - t'
    mask = idx >= 0
    U = w[:, np.clip(idx, 0, K - 1)] * mask  # (C, K, K): U[c, t, t'] = w[c, t-t']
    Wp = np.empty((128, (C // 2) * K), dtype=np.float32)
    # pair j: even channel 2j -> rows 0:64, odd channel 2j+1 -> rows 64:128
    Wp[0:64] = U[0::2].transpose(1, 0, 2).reshape(K, -1)
    Wp[64:128] = U[1::2].transpose(1, 0, 2).reshape(K, -1)
    return Wp


def kernel(x: np.ndarray, weight: np.ndarray) -> np.ndarray:
    import ml_dtypes
    from concourse.bass_utils import run_bass_kernel_spmd

    if MODE not in _cache:
        _cache[MODE] = _build_nc(mode=MODE)
    nc = _cache[MODE]

    x = np.ascontiguousarray(x, dtype=np.float32)
    Wp = _toeplitz_weights(np.asarray(weight))
    if MODE == "bf16sbd":
        Wbd = np.zeros((128, (C // 2) * 128), np.float32)
        for j in range(C // 2):
            Wbd[0:64, j * 128:j * 128 + 64] = Wp[0:64, j * K:(j + 1) * K]
            Wbd[64:128, j * 128 + 64:(j + 1) * 128] = Wp[64:128, j * K:(j + 1) * K]
        Wbdh = Wbd.astype(ml_dtypes.bfloat16)
        Wbdl = (Wbd - Wbdh.astype(np.float32)).astype(ml_dtypes.bfloat16)

    x_raw = x.reshape(NB, CT)
    in_maps = []
    for k in range(NCORES):
        m = {"x": x_raw[k * NPC:(k + 1) * NPC]}
        if MODE == "bf16sbd":
            m["w"] = Wbdh
            m["wl"] = Wbdl
        elif MODE in ("bf16s", "bf16sbdc"):
            Wh = Wp.astype(ml_dtypes.bfloat16)
            m["w"] = Wh
            m["wl"] = (Wp - Wh.astype(np.float32)).astype(ml_dtypes.bfloat16)
        else:
            m["w"] = Wp
        in_maps.append(m)
    res = run_bass_kernel_spmd(nc, in_maps, core_ids=list(range(NCORES)),
                               trace=TRACE)
    global LAST_RESULT
    LAST_RESULT = res

    # Assemble: core k holds out_v[b = k//2, :, :, m-half]
    out_v = np.empty((B, C, T, H * W), dtype=np.float32)
    for k in range(NCORES):
        yk = res.results[k]["y"].reshape(C, T, NPC)
        b, half = divmod(k, 2)
        out_v[b, :, :, half * NPC:(half + 1) * NPC] = yk
    return out_v.reshape(B, C, T, H, W)


if __name__ == "__main__":
    x = np.load("/tmp/x.npy")
    w = np.load("/tmp/w.npy")
    out = kernel(x, w)
    exp = np.load("/tmp/expected.npy")
    denom = np.abs(exp).max()
    print("max abs err:", np.abs(out - exp).max(), "absmax:", denom)
    print("rel:", np.abs(out - exp).max() / denom)
